# revision 46
# baseline (speedup 1.0000x reference)
"""GATv2Conv batched-graph kernel for Trainium2 (8 NeuronCores, data-parallel).

Problem: B=16384 independent 9-node graphs, C_in=C_out=256, fixed edge list
(16 directed tree edges + 9 self-loops = 25 edges), GATv2 attention.
Sharding: pure data parallel over the batch dim (2048 graphs per core),
params replicated; ~263us HW exec (baseline was 342us).

Design:
  - Custom fused DVE op LEAKY_ADD_ANT: st = max(xl+xr, 0.2*(xl+xr)) in one
    ~1.15 cyc/elem instruction (stock path was a tensor_tensor add plus a
    2.2 cyc/elem scalar_tensor_tensor or an ACT Prelu), registered into the
    ant custom-DVE table at import time.  All leaky-adds run on DVE.
  - Edges ordered self-loops-first then grouped by source node so the
    leaky-adds run as a few wide strided ops ([128, 9*G] for all
    self-loops at once; per-src groups use one strided dst AP + a
    stride-0 broadcast src AP).
  - Shallow cross-block pipeline: the PE stream per block is
    proj -> gm-proj -> scores -> den/transposes with the previous block's
    aggregation closures drained through the proj/gm loops, so the tensor
    engine stays busy (p-state) and the score matmuls' inputs are produced
    by DVE during the proj/gm phases.  xT is DMA-prefetched a block ahead.
  - All PSUM->SBUF copies on the scalar (ACT) engine; aggregation
    (graph-major fused mult-add chains, alpha as per-partition scalar) on
    DVE, round-robined across destination nodes to avoid RMW stalls;
    aggregation init (self-loop term) via ACT copy-with-scale.
  - PSUM accumulation-group gotcha: the two K-chunk matmuls of one output
    region must be emitted consecutively (j-outer, kc-inner); interleaving
    start/stop groups of different column regions of a bank corrupts the
    accumulation.
  - bias handled host-side (zeros in this problem).
"""

import sys

if "/opt/trn_rl_repo" not in sys.path:
    sys.path.insert(0, "/opt/trn_rl_repo")

import numpy as np
import ml_dtypes

import concourse.bass as bass
import concourse.bacc as bacc
import concourse.mybir as mybir
from concourse import tile
from concourse.bass_utils import run_bass_kernel_spmd

# ---- register the fused leaky-add custom DVE op ----
from concourse import dve_ops as _dops
from concourse.dve_spec import Spec as _Spec, Src0 as _S0, Src1 as _S1, \
    C0 as _C0, maxx as _maxx, lower as _lower
from concourse.dve_uop import DveOpSpec as _DveOpSpec

_LSPEC = _Spec(
    body=_maxx(_S0 + _S1, (_S0 + _S1) * _C0),
    reference=lambda in0, in1, s0, s1, imm2: np.maximum(
        in0 + in1, (in0 + in1) * s0),
)


def _register_leaky_add():
    if "LEAKY_ADD_ANT" in _dops._SUB_OPCODE_FOR_NAME:
        return next(op for op in _dops.OPS if op.name == "LEAKY_ADD_ANT")
    op = _dops.DveOp("LEAKY_ADD_ANT", _LSPEC, subdim=False, uops_sha={})
    for ver in ("v3", "v4"):
        try:
            sha = _DveOpSpec(
                name="LEAKY_ADD_ANT", opcode=0,
                uops=_lower(_LSPEC, ver=ver), rd1_en=True).sha(ver)
            op.uops_sha[ver] = sha
        except Exception:
            pass
    row = _dops._CUSTOM_DVE_ROW_BASE + len(_dops.OPS)
    assert row < 0x20
    _dops.OPS.append(op)
    _dops.CUSTOM_DVE_SPECS["LEAKY_ADD_ANT"] = _LSPEC
    _dops._SUB_OPCODE_FOR_NAME["LEAKY_ADD_ANT"] = row
    return op


LEAKY_ADD_ANT = _register_leaky_add()

_S2SPEC = _Spec(
    body=_S0 * _C0 + _S1 * __import__("concourse.dve_spec",
                                      fromlist=["C1"]).C1,
    reference=lambda in0, in1, s0, s1, imm2: in0 * s0 + in1 * s1,
)


def _register_scale2_add():
    if "SCALE2_ADD_ANT" in _dops._SUB_OPCODE_FOR_NAME:
        return next(op for op in _dops.OPS if op.name == "SCALE2_ADD_ANT")
    op = _dops.DveOp("SCALE2_ADD_ANT", _S2SPEC, subdim=False, uops_sha={})
    for ver in ("v3", "v4"):
        try:
            sha = _DveOpSpec(
                name="SCALE2_ADD_ANT", opcode=0,
                uops=_lower(_S2SPEC, ver=ver), rd1_en=True).sha(ver)
            op.uops_sha[ver] = sha
        except Exception:
            pass
    row = _dops._CUSTOM_DVE_ROW_BASE + len(_dops.OPS)
    assert row < 0x20
    _dops.OPS.append(op)
    _dops.CUSTOM_DVE_SPECS["SCALE2_ADD_ANT"] = _S2SPEC
    _dops._SUB_OPCODE_FOR_NAME["SCALE2_ADD_ANT"] = row
    return op


SCALE2_ADD_ANT = _register_scale2_add()

F32 = mybir.dt.float32
BF16 = mybir.dt.bfloat16

N_CORES = 8
B_TOTAL = 16384
NEG_SLOPE = 0.2
BC = B_TOTAL // N_CORES          # graphs per core
NN = 9                           # nodes per graph
C = 256                          # channels
G = 512                          # graphs per block
NBLK = BC // G                   # blocks per core
NT = G // 128                    # 128-graph subtiles per block
NGT = NN * G                     # columns per (chunk, block)

# ---- static edge list ----
# Order: 9 self-loops first (edge e = node e), then tree edges grouped by
# SOURCE node (dst lists are arithmetic sequences -> one strided AP each).
_ADJ = {0: [1, 3, 5, 7], 1: [0, 2], 2: [1], 3: [0, 4], 4: [3],
        5: [0, 6], 6: [5], 7: [0, 8], 8: [7]}
EDGES = [(d, d) for d in range(NN)]
SRC_GROUPS = []     # (src, [dsts], edge_base)
for _s in range(NN):
    SRC_GROUPS.append((_s, _ADJ[_s], len(EDGES)))
    for _d in _ADJ[_s]:
        EDGES.append((_s, _d))
NE = len(EDGES)     # 25
assert NE == 25
IN_EDGES = [[e for e, (_s, d) in enumerate(EDGES) if d == dd and _s == dd] +
            [e for e, (_s, d) in enumerate(EDGES) if d == dd and _s != dd]
            for dd in range(NN)]

# leaky-add ops: (in0_spec, in1_slice(start,stop,step), out_base, n).
# in0_spec: ("slice", (start,stop,step)) or ("bcast", src_node).
LADD_OPS = [(("slice", (0, 9, 1)), (0, 9, 1), 0, 9)] + [
    (("bcast", _s),
     (_dsts[0], _dsts[-1] + 1, (_dsts[1] - _dsts[0]) if len(_dsts) > 1
      else 1),
     _eb, len(_dsts))
    for (_s, _dsts, _eb) in SRC_GROUPS]


class Cfg:
    # per leaky-add unit (cycled): "dve" = fused custom op on DVE;
    # "pa" = tensor_tensor add on Pool + Prelu on ACT (per-edge 2-D ops)
    ladd_units = ("dve",) * 10
    agg_engines = ("affine",)          # "vector" (STT) | "affine" (custom)
    aggi_engines = ("scalar",)         # "scalar" | "affine" (zero trick)
    pcopy_engines = ("scalar",)
    gcopy_engines = ("scalar",)
    prev_per_unit = 3                  # prev-block agg closures per psum unit
    drain_skip = 5                     # proj units before draining starts
    score_ilv_start = 14               # gm unit where score interleave begins
    score_ilv_k = 4                    # score matmuls per gm unit
    pool_deg2 = True                   # deg-2 dst agg second edge on Pool


def build_program(cfg: Cfg):
    nc = bacc.Bacc("TRN2", target_bir_lowering=False, debug=False)

    def eng(name):
        return {"vector": nc.vector, "gpsimd": nc.gpsimd,
                "scalar": nc.scalar}[name]

    def copy_op(ename, dst_ap, src_ap):
        if ename == "scalar":
            nc.scalar.copy(dst_ap, src_ap)
        else:
            eng(ename).tensor_copy(dst_ap, src_ap)

    # DRAM tensors
    xT_d = nc.dram_tensor("xT", [C, NBLK * NGT], BF16, kind="ExternalInput")
    wl_d = nc.dram_tensor("wl", [C, C], BF16, kind="ExternalInput")
    wr_d = nc.dram_tensor("wr", [C, C], BF16, kind="ExternalInput")
    attbl_d = nc.dram_tensor("attbl", [128, 2 * NE * NE], BF16,
                             kind="ExternalInput")
    smat_d = nc.dram_tensor("smat", [NE, NE], F32, kind="ExternalInput")
    identf_d = nc.dram_tensor("identf", [128, 128], F32, kind="ExternalInput")
    out_d = nc.dram_tensor("out", [BC, NN * C], BF16, kind="ExternalOutput")

    with tile.TileContext(nc) as tc:
        with (
            tc.tile_pool(name="const", bufs=1) as cpool,
            tc.tile_pool(name="xin", bufs=3) as xpool,
            tc.tile_pool(name="proj", bufs=1) as prpool,
            tc.tile_pool(name="edge", bufs=1) as epool,
            tc.tile_pool(name="soft", bufs=2) as spool,
            tc.tile_pool(name="gm", bufs=2) as gmpool,
            tc.tile_pool(name="outp", bufs=2) as opool,
            tc.tile_pool(name="pair", bufs=2) as papool,
            tc.tile_pool(name="ps_proj", bufs=2, space="PSUM") as ps_proj,
            tc.tile_pool(name="ps_gm", bufs=2, space="PSUM") as ps_gm,
            tc.tile_pool(name="ps_sc", bufs=1, space="PSUM") as ps_sc,
        ):
            # ---- constants ----
            wl_sb = cpool.tile([128, 2 * C], BF16, tag="wl")
            wr_sb = cpool.tile([128, 2 * C], BF16, tag="wr")
            nc.sync.dma_start(wl_sb[:, 0:C], wl_d[0:128, :])
            nc.sync.dma_start(wl_sb[:, C:2 * C], wl_d[128:256, :])
            nc.sync.dma_start(wr_sb[:, 0:C], wr_d[0:128, :])
            nc.sync.dma_start(wr_sb[:, C:2 * C], wr_d[128:256, :])
            attbl_sb = cpool.tile([128, 2 * NE * NE], BF16, tag="attbl")
            nc.sync.dma_start(attbl_sb[:], attbl_d[:])
            smat_sb = cpool.tile([NE, NE], F32, tag="smat")
            nc.sync.dma_start(smat_sb[:], smat_d[:])
            identf_sb = cpool.tile([128, 128], F32, tag="identf")
            nc.sync.dma_start(identf_sb[:], identf_d[:])
            zero_sb = cpool.tile([128, C], BF16, tag="zero")
            nc.vector.memset(zero_sb[:], 0.0)

            ecyc = {}

            def cyc(kind):
                lst = getattr(cfg, kind + "_engines")
                i = ecyc.get(kind, 0)
                ecyc[kind] = i + 1
                return lst[i % len(lst)]

            def emit_edge_ops(pv, dch):
                """Fused leaky(xl+xr) producing this block's st tiles."""
                xlT, xrT = pv["xlT"], pv["xrT"]
                st = pv["st"]
                for (i0spec, i1sl, eb, n) in LADD_OPS:
                    a, b2, step = i1sl
                    in1 = xrT[dch][:, a:b2:step, :]
                    if i0spec[0] == "slice":
                        sa, sb, ss = i0spec[1]
                        in0 = xlT[dch][:, sa:sb:ss, :]
                    else:
                        s = i0spec[1]
                        in0 = xlT[dch][:, s:s + 1, :].broadcast_to(
                            [128, n, G])
                    nc.vector._custom_dve(
                        LEAKY_ADD_ANT, out=st[dch][:, eb:eb + n, :],
                        in0=in0, in1=in1, s0=NEG_SLOPE)

            def make_agg_closures(pv):
                """Aggregation + output DMA closures for prev block."""
                alphaT = pv["alphaT"]
                xl_gms = pv["xl_gms"]
                bb = pv["b"]
                work = []
                for t in range(NT):
                    xl_gm = xl_gms[t]
                    out_t = opool.tile([128, NN * C], BF16, tag="out_t")

                    def mk_op(d, i, t=t, xl_gm=xl_gm, out_t=out_t):
                        def f():
                            es = IN_EDGES[d]
                            e = es[i]
                            al = alphaT[:, t * NE + e:t * NE + e + 1]
                            dsl = out_t[:, d * C:(d + 1) * C]
                            ssl = xl_gm[:, EDGES[e][0] * C:
                                        (EDGES[e][0] + 1) * C]
                            if i == 0:
                                ie = cyc("aggi")
                                if ie == "scalar":
                                    nc.scalar.activation(
                                        dsl, ssl,
                                        mybir.ActivationFunctionType.Copy,
                                        scale=al)
                                else:
                                    nc.vector.affine_then_add(
                                        dsl, ssl, zero_sb[:], al, 0.0)
                            elif cfg.pool_deg2 and len(es) == 2:
                                msg = papool.tile([128, C], BF16,
                                                  tag=f"pm{d % 2}")
                                nc.gpsimd.tensor_tensor(
                                    msg[:], ssl,
                                    al.broadcast_to([128, C]),
                                    op=mybir.AluOpType.mult)
                                nc.gpsimd.tensor_tensor(
                                    dsl, msg[:], dsl,
                                    op=mybir.AluOpType.add)
                            else:
                                ge = cyc("agg")
                                if ge == "affine":
                                    nc.vector.affine_then_add(
                                        dsl, ssl, dsl, al, 0.0)
                                else:
                                    nc.vector.scalar_tensor_tensor(
                                        dsl, ssl, al, dsl,
                                        op0=mybir.AluOpType.mult,
                                        op1=mybir.AluOpType.add)
                        return f
                    # round-robin across dsts so consecutive DVE ops hit
                    # different out_t regions (no RMW pipeline stalls)
                    maxdeg = max(len(es) for es in IN_EDGES)
                    for i in range(maxdeg):
                        for d in range(NN):
                            if i < len(IN_EDGES[d]):
                                work.append(mk_op(d, i))

                    def dma(t=t, out_t=out_t):
                        nc.sync.dma_start(
                            out_d[bb * G + t * 128:
                                  bb * G + (t + 1) * 128, :],
                            out_t[:])
                    work.append(dma)
                return work

            prev = None
            xt_pre = []
            _first = []
            for chk in range(2):
                t_ = xpool.tile([128, NGT], BF16, tag=f"xt{chk}")
                nc.sync.dma_start(
                    t_[:], xT_d[chk * 128:(chk + 1) * 128, 0:NGT])
                _first.append(t_)
            xt_pre.append(_first)
            for b in range(NBLK + 1):
                cur = None
                pwork = make_agg_closures(prev) if prev is not None else []
                pi = 0

                def drain(k):
                    nonlocal pi
                    for _ in range(k):
                        if pi < len(pwork):
                            pwork[pi]()
                            pi += 1

                if b < NBLK:
                    cur = {"b": b}
                    xt = xt_pre.pop(0)
                    if b + 1 < NBLK:
                        nxt = []
                        for chk in range(2):
                            t_ = xpool.tile([128, NGT], BF16,
                                            tag=f"xt{chk}")
                            nc.sync.dma_start(
                                t_[:],
                                xT_d[chk * 128:(chk + 1) * 128,
                                     (b + 1) * NGT:(b + 2) * NGT])
                            nxt.append(t_)
                        xt_pre.append(nxt)

                    # ---- channel-major projections ----
                    xlT = [prpool.tile([128, NN, G], BF16, tag=f"xlT{d}",
                                       name=f"xlT{d}") for d in range(2)]
                    xrT = [prpool.tile([128, NN, G], BF16, tag=f"xrT{d}",
                                       name=f"xrT{d}") for d in range(2)]
                    cur["xlT"], cur["xrT"] = xlT, xrT
                    cur["st"] = [epool.tile([128, NE, G], BF16,
                                            tag=f"st{d}", name=f"st{d}")
                                 for d in range(2)]
                    for (wsb, dest, dch) in ((wl_sb, xlT, 0),
                                             (wr_sb, xrT, 0),
                                             (wl_sb, xlT, 1),
                                             (wr_sb, xrT, 1)):
                        if True:
                            s = 0
                            while s < NN:
                                npair = min(2, NN - s)
                                ps = ps_proj.tile([128, 2, G], F32,
                                                  tag="ps_proj")
                                for j in range(npair):
                                    for kc in range(2):
                                        nc.tensor.matmul(
                                            ps[:, j, :],
                                            wsb[:, kc * C + dch * 128:
                                                kc * C + dch * 128 + 128],
                                            xt[kc][:, (s + j) * G:
                                                   (s + j + 1) * G],
                                            start=(kc == 0), stop=(kc == 1))
                                copy_op(cyc("pcopy"),
                                        dest[dch][:, s:s + npair, :],
                                        ps[:, 0:npair, :])
                                ui_d = ecyc.get("_ui", 0)
                                ecyc["_ui"] = ui_d + 1
                                if ui_d >= cfg.drain_skip:
                                    drain(cfg.prev_per_unit)
                                s += npair
                        if wsb is wr_sb:
                            emit_edge_ops(cur, dch)

                    # ---- graph-major xl projection, with this block's
                    # dch0 score matmuls interleaved into the tail ----
                    st = cur["st"]
                    sc_ps = ps_sc.tile([NE, G], F32, tag="sc")
                    sc_order = [(e, dch) for dch in range(2)
                                for e in range(NE)]
                    si = 0

                    def emit_scores(k):
                        nonlocal si
                        for _ in range(k):
                            if si < len(sc_order):
                                e, dch = sc_order[si]
                                blk = (dch * NE + e) * NE
                                nc.tensor.matmul(
                                    sc_ps[:], attbl_sb[:, blk:blk + NE],
                                    st[dch][:, e, :],
                                    start=(e == 0 and dch == 0),
                                    stop=(e == NE - 1 and dch == 1))
                                si += 1

                    xl_gms = [gmpool.tile([128, NN * C], BF16,
                                          tag=f"xl_gm{t}", name=f"xl_gm{t}")
                              for t in range(NT)]
                    cur["xl_gms"] = xl_gms
                    gu = 0
                    for t in range(NT):
                        s = 0
                        while s < NN:
                            npair = min(2, NN - s)
                            ps = ps_gm.tile([128, 512], F32, tag="ps_gm")
                            for j in range(npair):
                                for kc in range(2):
                                    nc.tensor.matmul(
                                        ps[:, j * C:(j + 1) * C],
                                        xt[kc][:, (s + j) * G + t * 128:
                                               (s + j) * G + (t + 1) * 128],
                                        wl_sb[:, kc * C:(kc + 1) * C],
                                        start=(kc == 0), stop=(kc == 1))
                            copy_op(cyc("gcopy"),
                                    xl_gms[t][:, s * C:(s + npair) * C],
                                    ps[:, 0:npair * C])
                            drain(cfg.prev_per_unit)
                            if gu >= cfg.score_ilv_start:
                                emit_scores(cfg.score_ilv_k)
                            gu += 1
                            s += npair

                    drain(len(pwork))
                    emit_scores(len(sc_order))
                    ex_sb = spool.tile([NE, G], F32, tag="ex")
                    nc.scalar.activation(ex_sb[:], sc_ps[:],
                                         mybir.ActivationFunctionType.Exp)
                    mis1 = ps_sc.tile([128, G], F32, tag="mis")
                    den_ps = mis1[0:NE, 0:G]
                    nc.tensor.matmul(den_ps, smat_sb[:], ex_sb[:],
                                     start=True, stop=True)
                    den_sb = spool.tile([NE, G], F32, tag="den")
                    nc.scalar.copy(den_sb[:], den_ps)
                    exT_ps = ps_sc.tile([128, G], F32, tag="mis")
                    dT0 = NT * NE
                    for t in range(NT):
                        nc.tensor.transpose(
                            exT_ps[:, t * NE:(t + 1) * NE],
                            ex_sb[:, t * 128:(t + 1) * 128],
                            identf_sb[0:NE, 0:NE])
                        nc.tensor.transpose(
                            exT_ps[:, dT0 + t * NE:dT0 + (t + 1) * NE],
                            den_sb[:, t * 128:(t + 1) * 128],
                            identf_sb[0:NE, 0:NE])
                    rdenT = spool.tile([128, NT * NE], F32, tag="rdenT")
                    nc.vector.reciprocal(rdenT[:],
                                         exT_ps[:, dT0:dT0 + dT0])
                    alphaT = spool.tile([128, NT * NE], F32, tag="alphaT")
                    nc.vector.tensor_tensor(
                        alphaT[:], exT_ps[:, 0:dT0], rdenT[:],
                        op=mybir.AluOpType.mult)
                    cur["alphaT"] = alphaT
                else:
                    drain(len(pwork))

                prev = cur

    nc.compile()
    return nc


def make_host_inputs(x, W_l, W_r, att, cfg: Cfg):
    """Builds the per-core input maps (host-side sharding + layout prep)."""
    x = np.asarray(x, dtype=np.float32)
    W_l = np.ascontiguousarray(np.asarray(W_l, dtype=np.float32))
    W_r = np.ascontiguousarray(np.asarray(W_r, dtype=np.float32))
    att = np.asarray(att, dtype=np.float32)
    bf = ml_dtypes.bfloat16

    attbl = np.zeros((128, 2, NE, NE), dtype=np.float32)
    for dch in range(2):
        for e in range(NE):
            attbl[:, dch, e, e] = att[dch * 128:(dch + 1) * 128]
    attbl = attbl.reshape(128, 2 * NE * NE).astype(bf)

    smat = np.zeros((NE, NE), dtype=np.float32)
    for e1, (_s1, d1) in enumerate(EDGES):
        for e2, (_s2, d2) in enumerate(EDGES):
            if d1 == d2:
                smat[e1, e2] = 1.0

    ident = np.eye(128, dtype=np.float32)

    in_maps = []
    for c in range(N_CORES):
        xc = x[c * BC:(c + 1) * BC]                       # [BC, 9, 256]
        xT = np.ascontiguousarray(
            xc.reshape(NBLK, G, NN, C).transpose(3, 0, 2, 1).reshape(
                C, NBLK * NGT).astype(bf))
        in_maps.append({
            "xT": xT,
            "wl": W_l.astype(bf),
            "wr": W_r.astype(bf),
            "attbl": attbl,
            "smat": smat,
            "identf": ident,
        })
    return in_maps


_CACHE = {}


def _cfg_key(cfg: Cfg):
    return (cfg.ladd_units, cfg.agg_engines, cfg.aggi_engines,
            cfg.pcopy_engines, cfg.gcopy_engines, cfg.prev_per_unit,
            cfg.drain_skip, cfg.score_ilv_start, cfg.score_ilv_k,
            cfg.pool_deg2)


def _get_program(cfg: Cfg):
    key = _cfg_key(cfg)
    if key not in _CACHE:
        _CACHE[key] = build_program(cfg)
    return _CACHE[key]


def kernel(x, W_l, W_r, att, bias, cfg: Cfg = None, trace: bool = False,
           _results_holder: dict = None, **run_kwargs):
    cfg = cfg or Cfg()
    nc = _get_program(cfg)
    in_maps = make_host_inputs(x, W_l, W_r, att, cfg)
    res = run_bass_kernel_spmd(nc, in_maps, core_ids=list(range(N_CORES)),
                               trace=trace, **run_kwargs)
    if _results_holder is not None:
        _results_holder["res"] = res
    outs = [np.asarray(r["out"], dtype=np.float32).reshape(BC, NN, C)
            for r in res.results]
    out = np.concatenate(outs, axis=0)
    bias = np.asarray(bias, dtype=np.float32)
    if np.any(bias):
        out = out + bias
    return out.astype(np.float32)


# revision 47
# speedup vs baseline: 1.1128x; 1.1128x over previous
"""GATv2Conv batched-graph kernel for Trainium2 (8 NeuronCores, data-parallel).

Problem: B=16384 independent 9-node graphs, C_in=C_out=256, fixed edge list
(16 directed tree edges + 9 self-loops = 25 edges), GATv2 attention.
Sharding: pure data parallel over the batch dim (2048 graphs per core),
params replicated; ~263us HW exec (baseline was 342us).

Design:
  - Custom fused DVE op LEAKY_ADD_ANT: st = max(xl+xr, 0.2*(xl+xr)) in one
    ~1.15 cyc/elem instruction (stock path was a tensor_tensor add plus a
    2.2 cyc/elem scalar_tensor_tensor or an ACT Prelu), registered into the
    ant custom-DVE table at import time.  All leaky-adds run on DVE.
  - Edges ordered self-loops-first then grouped by source node so the
    leaky-adds run as a few wide strided ops ([128, 9*G] for all
    self-loops at once; per-src groups use one strided dst AP + a
    stride-0 broadcast src AP).
  - Shallow cross-block pipeline: the PE stream per block is
    proj -> gm-proj -> scores -> den/transposes with the previous block's
    aggregation closures drained through the proj/gm loops, so the tensor
    engine stays busy (p-state) and the score matmuls' inputs are produced
    by DVE during the proj/gm phases.  xT is DMA-prefetched a block ahead.
  - All PSUM->SBUF copies on the scalar (ACT) engine; aggregation
    (graph-major fused mult-add chains, alpha as per-partition scalar) on
    DVE, round-robined across destination nodes to avoid RMW stalls;
    aggregation init (self-loop term) via ACT copy-with-scale.
  - PSUM accumulation-group gotcha: the two K-chunk matmuls of one output
    region must be emitted consecutively (j-outer, kc-inner); interleaving
    start/stop groups of different column regions of a bank corrupts the
    accumulation.
  - bias handled host-side (zeros in this problem).
"""

import sys

if "/opt/trn_rl_repo" not in sys.path:
    sys.path.insert(0, "/opt/trn_rl_repo")

import numpy as np
import ml_dtypes

import concourse.bass as bass
import concourse.bacc as bacc
import concourse.mybir as mybir
from concourse import tile
from concourse.bass_utils import run_bass_kernel_spmd

# ---- register the fused leaky-add custom DVE op ----
from concourse import dve_ops as _dops
from concourse.dve_spec import Spec as _Spec, Src0 as _S0, Src1 as _S1, \
    C0 as _C0, maxx as _maxx, lower as _lower
from concourse.dve_uop import DveOpSpec as _DveOpSpec

_LSPEC = _Spec(
    body=_maxx(_S0 + _S1, (_S0 + _S1) * _C0),
    reference=lambda in0, in1, s0, s1, imm2: np.maximum(
        in0 + in1, (in0 + in1) * s0),
)


def _register_leaky_add():
    if "LEAKY_ADD_ANT" in _dops._SUB_OPCODE_FOR_NAME:
        return next(op for op in _dops.OPS if op.name == "LEAKY_ADD_ANT")
    op = _dops.DveOp("LEAKY_ADD_ANT", _LSPEC, subdim=False, uops_sha={})
    for ver in ("v3", "v4"):
        try:
            sha = _DveOpSpec(
                name="LEAKY_ADD_ANT", opcode=0,
                uops=_lower(_LSPEC, ver=ver), rd1_en=True).sha(ver)
            op.uops_sha[ver] = sha
        except Exception:
            pass
    row = _dops._CUSTOM_DVE_ROW_BASE + len(_dops.OPS)
    assert row < 0x20
    _dops.OPS.append(op)
    _dops.CUSTOM_DVE_SPECS["LEAKY_ADD_ANT"] = _LSPEC
    _dops._SUB_OPCODE_FOR_NAME["LEAKY_ADD_ANT"] = row
    return op


LEAKY_ADD_ANT = _register_leaky_add()

_S2SPEC = _Spec(
    body=_S0 * _C0 + _S1 * __import__("concourse.dve_spec",
                                      fromlist=["C1"]).C1,
    reference=lambda in0, in1, s0, s1, imm2: in0 * s0 + in1 * s1,
)


def _register_scale2_add():
    if "SCALE2_ADD_ANT" in _dops._SUB_OPCODE_FOR_NAME:
        return next(op for op in _dops.OPS if op.name == "SCALE2_ADD_ANT")
    op = _dops.DveOp("SCALE2_ADD_ANT", _S2SPEC, subdim=False, uops_sha={})
    for ver in ("v3", "v4"):
        try:
            sha = _DveOpSpec(
                name="SCALE2_ADD_ANT", opcode=0,
                uops=_lower(_S2SPEC, ver=ver), rd1_en=True).sha(ver)
            op.uops_sha[ver] = sha
        except Exception:
            pass
    row = _dops._CUSTOM_DVE_ROW_BASE + len(_dops.OPS)
    assert row < 0x20
    _dops.OPS.append(op)
    _dops.CUSTOM_DVE_SPECS["SCALE2_ADD_ANT"] = _S2SPEC
    _dops._SUB_OPCODE_FOR_NAME["SCALE2_ADD_ANT"] = row
    return op


SCALE2_ADD_ANT = _register_scale2_add()

F32 = mybir.dt.float32
BF16 = mybir.dt.bfloat16

N_CORES = 8
B_TOTAL = 16384
NEG_SLOPE = 0.2
BC = B_TOTAL // N_CORES          # graphs per core
NN = 9                           # nodes per graph
C = 256                          # channels
G = 512                          # graphs per block
NBLK = BC // G                   # blocks per core
NT = G // 128                    # 128-graph subtiles per block
NGT = NN * G                     # columns per (chunk, block)

# ---- static edge list ----
# Order: 9 self-loops first (edge e = node e), then tree edges grouped by
# SOURCE node (dst lists are arithmetic sequences -> one strided AP each).
_ADJ = {0: [1, 3, 5, 7], 1: [0, 2], 2: [1], 3: [0, 4], 4: [3],
        5: [0, 6], 6: [5], 7: [0, 8], 8: [7]}
EDGES = [(d, d) for d in range(NN)]
SRC_GROUPS = []     # (src, [dsts], edge_base)
for _s in range(NN):
    SRC_GROUPS.append((_s, _ADJ[_s], len(EDGES)))
    for _d in _ADJ[_s]:
        EDGES.append((_s, _d))
NE = len(EDGES)     # 25
assert NE == 25
IN_EDGES = [[e for e, (_s, d) in enumerate(EDGES) if d == dd and _s == dd] +
            [e for e, (_s, d) in enumerate(EDGES) if d == dd and _s != dd]
            for dd in range(NN)]

# leaky-add ops: (in0_spec, in1_slice(start,stop,step), out_base, n).
# in0_spec: ("slice", (start,stop,step)) or ("bcast", src_node).
LADD_OPS = [(("slice", (0, 9, 1)), (0, 9, 1), 0, 9)] + [
    (("bcast", _s),
     (_dsts[0], _dsts[-1] + 1, (_dsts[1] - _dsts[0]) if len(_dsts) > 1
      else 1),
     _eb, len(_dsts))
    for (_s, _dsts, _eb) in SRC_GROUPS]


class Cfg:
    # per leaky-add unit (cycled): "dve" = fused custom op on DVE;
    # "pa" = tensor_tensor add on Pool + Prelu on ACT (per-edge 2-D ops)
    ladd_units = ("dve",) * 10
    agg_engines = ("affine",)          # "vector" (STT) | "affine" (custom)
    aggi_engines = ("scalar",)         # "scalar" | "affine" (zero trick)
    pcopy_engines = ("scalar",)
    gcopy_engines = ("scalar",)
    prev_per_unit = 3                  # prev-block agg closures per psum unit
    drain_skip = 5                     # proj units before draining starts
    score_ilv_start = 14               # gm unit where score interleave begins
    score_ilv_k = 4                    # score matmuls per gm unit
    pool_deg2 = False                  # deg-2 dst agg on Pool: slower


def build_program(cfg: Cfg):
    nc = bacc.Bacc("TRN2", target_bir_lowering=False, debug=False)

    def eng(name):
        return {"vector": nc.vector, "gpsimd": nc.gpsimd,
                "scalar": nc.scalar}[name]

    def copy_op(ename, dst_ap, src_ap):
        if ename == "scalar":
            nc.scalar.copy(dst_ap, src_ap)
        else:
            eng(ename).tensor_copy(dst_ap, src_ap)

    # DRAM tensors
    xT_d = nc.dram_tensor("xT", [C, NBLK * NGT], BF16, kind="ExternalInput")
    wl_d = nc.dram_tensor("wl", [C, C], BF16, kind="ExternalInput")
    wr_d = nc.dram_tensor("wr", [C, C], BF16, kind="ExternalInput")
    attbl_d = nc.dram_tensor("attbl", [128, 2 * NE * NE], BF16,
                             kind="ExternalInput")
    smat_d = nc.dram_tensor("smat", [NE, NE], F32, kind="ExternalInput")
    identf_d = nc.dram_tensor("identf", [128, 128], F32, kind="ExternalInput")
    out_d = nc.dram_tensor("out", [BC, NN * C], BF16, kind="ExternalOutput")

    with tile.TileContext(nc) as tc:
        with (
            tc.tile_pool(name="const", bufs=1) as cpool,
            tc.tile_pool(name="xin", bufs=3) as xpool,
            tc.tile_pool(name="proj", bufs=1) as prpool,
            tc.tile_pool(name="edge", bufs=1) as epool,
            tc.tile_pool(name="soft", bufs=2) as spool,
            tc.tile_pool(name="gm", bufs=2) as gmpool,
            tc.tile_pool(name="outp", bufs=2) as opool,
            tc.tile_pool(name="pair", bufs=2) as papool,
            tc.tile_pool(name="ps_proj", bufs=2, space="PSUM") as ps_proj,
            tc.tile_pool(name="ps_gm", bufs=2, space="PSUM") as ps_gm,
            tc.tile_pool(name="ps_sc", bufs=1, space="PSUM") as ps_sc,
        ):
            # ---- constants ----
            wl_sb = cpool.tile([128, 2 * C], BF16, tag="wl")
            wr_sb = cpool.tile([128, 2 * C], BF16, tag="wr")
            nc.sync.dma_start(wl_sb[:, 0:C], wl_d[0:128, :])
            nc.sync.dma_start(wl_sb[:, C:2 * C], wl_d[128:256, :])
            nc.sync.dma_start(wr_sb[:, 0:C], wr_d[0:128, :])
            nc.sync.dma_start(wr_sb[:, C:2 * C], wr_d[128:256, :])
            attbl_sb = cpool.tile([128, 2 * NE * NE], BF16, tag="attbl")
            nc.sync.dma_start(attbl_sb[:], attbl_d[:])
            smat_sb = cpool.tile([NE, NE], F32, tag="smat")
            nc.sync.dma_start(smat_sb[:], smat_d[:])
            identf_sb = cpool.tile([128, 128], F32, tag="identf")
            nc.sync.dma_start(identf_sb[:], identf_d[:])
            zero_sb = cpool.tile([128, C], BF16, tag="zero")
            nc.vector.memset(zero_sb[:], 0.0)

            ecyc = {}

            def cyc(kind):
                lst = getattr(cfg, kind + "_engines")
                i = ecyc.get(kind, 0)
                ecyc[kind] = i + 1
                return lst[i % len(lst)]

            def emit_edge_ops(pv, dch):
                """Fused leaky(xl+xr) producing this block's st tiles."""
                xlT, xrT = pv["xlT"], pv["xrT"]
                st = pv["st"]
                for (i0spec, i1sl, eb, n) in LADD_OPS:
                    a, b2, step = i1sl
                    in1 = xrT[dch][:, a:b2:step, :]
                    if i0spec[0] == "slice":
                        sa, sb, ss = i0spec[1]
                        in0 = xlT[dch][:, sa:sb:ss, :]
                    else:
                        s = i0spec[1]
                        in0 = xlT[dch][:, s:s + 1, :].broadcast_to(
                            [128, n, G])
                    nc.vector._custom_dve(
                        LEAKY_ADD_ANT, out=st[dch][:, eb:eb + n, :],
                        in0=in0, in1=in1, s0=NEG_SLOPE)

            def make_agg_closures(pv):
                """Aggregation + output DMA closures for prev block."""
                alphaT = pv["alphaT"]
                xl_gms = pv["xl_gms"]
                bb = pv["b"]
                work = []
                for t in range(NT):
                    xl_gm = xl_gms[t]
                    out_t = opool.tile([128, NN * C], BF16, tag="out_t")

                    def mk_op(d, i, t=t, xl_gm=xl_gm, out_t=out_t):
                        def f():
                            es = IN_EDGES[d]
                            e = es[i]
                            al = alphaT[:, t * NE + e:t * NE + e + 1]
                            dsl = out_t[:, d * C:(d + 1) * C]
                            ssl = xl_gm[:, EDGES[e][0] * C:
                                        (EDGES[e][0] + 1) * C]
                            if i == 0:
                                ie = cyc("aggi")
                                if ie == "scalar":
                                    nc.scalar.activation(
                                        dsl, ssl,
                                        mybir.ActivationFunctionType.Copy,
                                        scale=al)
                                else:
                                    nc.vector.affine_then_add(
                                        dsl, ssl, zero_sb[:], al, 0.0)
                            elif cfg.pool_deg2 and len(es) == 2:
                                msg = papool.tile([128, C], BF16,
                                                  tag=f"pm{d % 2}")
                                nc.gpsimd.tensor_tensor(
                                    msg[:], ssl,
                                    al.broadcast_to([128, C]),
                                    op=mybir.AluOpType.mult)
                                nc.gpsimd.tensor_tensor(
                                    dsl, msg[:], dsl,
                                    op=mybir.AluOpType.add)
                            else:
                                ge = cyc("agg")
                                if ge == "affine":
                                    nc.vector.affine_then_add(
                                        dsl, ssl, dsl, al, 0.0)
                                else:
                                    nc.vector.scalar_tensor_tensor(
                                        dsl, ssl, al, dsl,
                                        op0=mybir.AluOpType.mult,
                                        op1=mybir.AluOpType.add)
                        return f
                    # round-robin across dsts so consecutive DVE ops hit
                    # different out_t regions (no RMW pipeline stalls)
                    maxdeg = max(len(es) for es in IN_EDGES)
                    for i in range(maxdeg):
                        for d in range(NN):
                            if i < len(IN_EDGES[d]):
                                work.append(mk_op(d, i))

                    def dma(t=t, out_t=out_t):
                        nc.sync.dma_start(
                            out_d[bb * G + t * 128:
                                  bb * G + (t + 1) * 128, :],
                            out_t[:])
                    work.append(dma)
                return work

            prev = None
            xt_pre = []
            _first = []
            for chk in range(2):
                t_ = xpool.tile([128, NGT], BF16, tag=f"xt{chk}")
                nc.sync.dma_start(
                    t_[:], xT_d[chk * 128:(chk + 1) * 128, 0:NGT])
                _first.append(t_)
            xt_pre.append(_first)
            for b in range(NBLK + 1):
                cur = None
                pwork = make_agg_closures(prev) if prev is not None else []
                pi = 0

                def drain(k):
                    nonlocal pi
                    for _ in range(k):
                        if pi < len(pwork):
                            pwork[pi]()
                            pi += 1

                if b < NBLK:
                    cur = {"b": b}
                    xt = xt_pre.pop(0)
                    if b + 1 < NBLK:
                        nxt = []
                        for chk in range(2):
                            t_ = xpool.tile([128, NGT], BF16,
                                            tag=f"xt{chk}")
                            nc.sync.dma_start(
                                t_[:],
                                xT_d[chk * 128:(chk + 1) * 128,
                                     (b + 1) * NGT:(b + 2) * NGT])
                            nxt.append(t_)
                        xt_pre.append(nxt)

                    # ---- channel-major projections ----
                    xlT = [prpool.tile([128, NN, G], BF16, tag=f"xlT{d}",
                                       name=f"xlT{d}") for d in range(2)]
                    xrT = [prpool.tile([128, NN, G], BF16, tag=f"xrT{d}",
                                       name=f"xrT{d}") for d in range(2)]
                    cur["xlT"], cur["xrT"] = xlT, xrT
                    cur["st"] = [epool.tile([128, NE, G], BF16,
                                            tag=f"st{d}", name=f"st{d}")
                                 for d in range(2)]
                    for (wsb, dest, dch) in ((wl_sb, xlT, 0),
                                             (wr_sb, xrT, 0),
                                             (wl_sb, xlT, 1),
                                             (wr_sb, xrT, 1)):
                        if True:
                            s = 0
                            while s < NN:
                                npair = min(2, NN - s)
                                ps = ps_proj.tile([128, 2, G], F32,
                                                  tag="ps_proj")
                                for j in range(npair):
                                    for kc in range(2):
                                        nc.tensor.matmul(
                                            ps[:, j, :],
                                            wsb[:, kc * C + dch * 128:
                                                kc * C + dch * 128 + 128],
                                            xt[kc][:, (s + j) * G:
                                                   (s + j + 1) * G],
                                            start=(kc == 0), stop=(kc == 1))
                                copy_op(cyc("pcopy"),
                                        dest[dch][:, s:s + npair, :],
                                        ps[:, 0:npair, :])
                                ui_d = ecyc.get("_ui", 0)
                                ecyc["_ui"] = ui_d + 1
                                if ui_d >= cfg.drain_skip:
                                    drain(cfg.prev_per_unit)
                                s += npair
                        if wsb is wr_sb:
                            emit_edge_ops(cur, dch)

                    # ---- graph-major xl projection, with this block's
                    # dch0 score matmuls interleaved into the tail ----
                    st = cur["st"]
                    sc_ps = ps_sc.tile([NE, G], F32, tag="sc")
                    sc_order = [(e, dch) for dch in range(2)
                                for e in range(NE)]
                    si = 0

                    def emit_scores(k):
                        nonlocal si
                        for _ in range(k):
                            if si < len(sc_order):
                                e, dch = sc_order[si]
                                blk = (dch * NE + e) * NE
                                nc.tensor.matmul(
                                    sc_ps[:], attbl_sb[:, blk:blk + NE],
                                    st[dch][:, e, :],
                                    start=(e == 0 and dch == 0),
                                    stop=(e == NE - 1 and dch == 1))
                                si += 1

                    xl_gms = [gmpool.tile([128, NN * C], BF16,
                                          tag=f"xl_gm{t}", name=f"xl_gm{t}")
                              for t in range(NT)]
                    cur["xl_gms"] = xl_gms
                    gu = 0
                    for t in range(NT):
                        s = 0
                        while s < NN:
                            npair = min(2, NN - s)
                            ps = ps_gm.tile([128, 512], F32, tag="ps_gm")
                            for j in range(npair):
                                for kc in range(2):
                                    nc.tensor.matmul(
                                        ps[:, j * C:(j + 1) * C],
                                        xt[kc][:, (s + j) * G + t * 128:
                                               (s + j) * G + (t + 1) * 128],
                                        wl_sb[:, kc * C:(kc + 1) * C],
                                        start=(kc == 0), stop=(kc == 1))
                            copy_op(cyc("gcopy"),
                                    xl_gms[t][:, s * C:(s + npair) * C],
                                    ps[:, 0:npair * C])
                            drain(cfg.prev_per_unit)
                            if gu >= cfg.score_ilv_start:
                                emit_scores(cfg.score_ilv_k)
                            gu += 1
                            s += npair

                    drain(len(pwork))
                    emit_scores(len(sc_order))
                    ex_sb = spool.tile([NE, G], F32, tag="ex")
                    nc.scalar.activation(ex_sb[:], sc_ps[:],
                                         mybir.ActivationFunctionType.Exp)
                    mis1 = ps_sc.tile([128, G], F32, tag="mis")
                    den_ps = mis1[0:NE, 0:G]
                    nc.tensor.matmul(den_ps, smat_sb[:], ex_sb[:],
                                     start=True, stop=True)
                    den_sb = spool.tile([NE, G], F32, tag="den")
                    nc.scalar.copy(den_sb[:], den_ps)
                    exT_ps = ps_sc.tile([128, G], F32, tag="mis")
                    dT0 = NT * NE
                    for t in range(NT):
                        nc.tensor.transpose(
                            exT_ps[:, t * NE:(t + 1) * NE],
                            ex_sb[:, t * 128:(t + 1) * 128],
                            identf_sb[0:NE, 0:NE])
                        nc.tensor.transpose(
                            exT_ps[:, dT0 + t * NE:dT0 + (t + 1) * NE],
                            den_sb[:, t * 128:(t + 1) * 128],
                            identf_sb[0:NE, 0:NE])
                    rdenT = spool.tile([128, NT * NE], F32, tag="rdenT")
                    nc.vector.reciprocal(rdenT[:],
                                         exT_ps[:, dT0:dT0 + dT0])
                    alphaT = spool.tile([128, NT * NE], F32, tag="alphaT")
                    nc.vector.tensor_tensor(
                        alphaT[:], exT_ps[:, 0:dT0], rdenT[:],
                        op=mybir.AluOpType.mult)
                    cur["alphaT"] = alphaT
                else:
                    drain(len(pwork))

                prev = cur

    nc.compile()
    return nc


def make_host_inputs(x, W_l, W_r, att, cfg: Cfg):
    """Builds the per-core input maps (host-side sharding + layout prep)."""
    x = np.asarray(x, dtype=np.float32)
    W_l = np.ascontiguousarray(np.asarray(W_l, dtype=np.float32))
    W_r = np.ascontiguousarray(np.asarray(W_r, dtype=np.float32))
    att = np.asarray(att, dtype=np.float32)
    bf = ml_dtypes.bfloat16

    attbl = np.zeros((128, 2, NE, NE), dtype=np.float32)
    for dch in range(2):
        for e in range(NE):
            attbl[:, dch, e, e] = att[dch * 128:(dch + 1) * 128]
    attbl = attbl.reshape(128, 2 * NE * NE).astype(bf)

    smat = np.zeros((NE, NE), dtype=np.float32)
    for e1, (_s1, d1) in enumerate(EDGES):
        for e2, (_s2, d2) in enumerate(EDGES):
            if d1 == d2:
                smat[e1, e2] = 1.0

    ident = np.eye(128, dtype=np.float32)

    in_maps = []
    for c in range(N_CORES):
        xc = x[c * BC:(c + 1) * BC]                       # [BC, 9, 256]
        xT = np.ascontiguousarray(
            xc.reshape(NBLK, G, NN, C).transpose(3, 0, 2, 1).reshape(
                C, NBLK * NGT).astype(bf))
        in_maps.append({
            "xT": xT,
            "wl": W_l.astype(bf),
            "wr": W_r.astype(bf),
            "attbl": attbl,
            "smat": smat,
            "identf": ident,
        })
    return in_maps


_CACHE = {}


def _cfg_key(cfg: Cfg):
    return (cfg.ladd_units, cfg.agg_engines, cfg.aggi_engines,
            cfg.pcopy_engines, cfg.gcopy_engines, cfg.prev_per_unit,
            cfg.drain_skip, cfg.score_ilv_start, cfg.score_ilv_k,
            cfg.pool_deg2)


def _get_program(cfg: Cfg):
    key = _cfg_key(cfg)
    if key not in _CACHE:
        _CACHE[key] = build_program(cfg)
    return _CACHE[key]


def kernel(x, W_l, W_r, att, bias, cfg: Cfg = None, trace: bool = False,
           _results_holder: dict = None, **run_kwargs):
    cfg = cfg or Cfg()
    nc = _get_program(cfg)
    in_maps = make_host_inputs(x, W_l, W_r, att, cfg)
    res = run_bass_kernel_spmd(nc, in_maps, core_ids=list(range(N_CORES)),
                               trace=trace, **run_kwargs)
    if _results_holder is not None:
        _results_holder["res"] = res
    outs = [np.asarray(r["out"], dtype=np.float32).reshape(BC, NN, C)
            for r in res.results]
    out = np.concatenate(outs, axis=0)
    bias = np.asarray(bias, dtype=np.float32)
    if np.any(bias):
        out = out + bias
    return out.astype(np.float32)
